# revision 1
# baseline (speedup 1.0000x reference)
"""Trainium2 Bass kernel for nn_Estor_concat (scatter_memory).

Math (exact reformulation of the reference):
  v_tag = (tag_emb @ Wv.T + bv) @ out_proj_w.T + out_proj_b        [T, H]
  W_eff[t, j] = sum_h v_tag[t, h] * ff1_w[j, t*H + h]              [T, H]
  counts[t, s] = #spans(tag t) covering s   (PE matmul over span masks)
  h1 = relu(W_eff.T @ counts + ff1_b)       (bias via K=17 matmul row)
  h2 = ff2 @ relu(h1) + ff2_b  (needed only for its per-position sumsq)
  x = [word_emb | h2]; LayerNorm folded into the output projection:
  out[l,s] = raw[l,s]*rstd[s] + c1n[l]*(mu*rstd)[s] + d[l]*rstd[s] + c2[l]
  raw = lwg_we.T @ we + G.T @ relu  with  G = ff2.T @ lwg_h2  (host-folded)

All per-tag constants (v_tag, W_eff) and weight-only products (lwg =
lin_w.T*g, G, c1n, d, c2) are folded on the host.  The device kernel does
the data-dependent work: span scatter -> counts, h1/h2, the raw/sum/sumsq
projections, LN stats and the output projection.  Sharding: pure
data-parallel over batch (8 cores, 1 batch each) - no collectives.
fp8 DoubleRow matmuls carry the h2/sumsq and G paths; scale factors
32 (ff2) and 16 (G, with relu emitting r/16) keep fp8 operands in the
normal range and cancel exactly in the psum accumulations.
"""

import ml_dtypes
import numpy as np

import concourse.bacc as bacc
import concourse.bass as bass
import concourse.mybir as mybir
import concourse.tile as tile
from concourse.bass_utils import run_bass_kernel_spmd

T, B, S, H = 16, 8, 512, 768
H2 = 384
NEW_H = H + H2          # 1152
NL = 33                 # num labels
EPS = 1e-12
NCORES = 8
P = 128
KC_H = H // 128         # 6 chunks of the hidden dim
KC_H2 = H2 // 128       # 3
MPR = 65                # psum rows: raw 0:33, zeros, sum row at 64
GSC = 16.0              # fp8 scale on G (relu emits r/16)
FSC = 32.0              # fp8 scale on ff2.T
W8 = 512                # padded pair width in the fp8 block (>= H2 + NPR)

F32 = mybir.dt.float32
BF16 = mybir.dt.bfloat16
F16 = mybir.dt.float16
F8 = mybir.dt.float8e4
AF = mybir.ActivationFunctionType
ALU = mybir.AluOpType


def build_kernel(n_span_tiles: int):
    nc = bacc.Bacc(
        "TRN2",
        target_bir_lowering=False,
        debug=False,
        enable_asserts=True,
        num_devices=NCORES,
    )

    def inp(name, shape, dtype=F32):
        return nc.dram_tensor(name, list(shape), dtype, kind="ExternalInput").ap()

    NT = n_span_tiles
    CF16 = S + T                          # iota_s | iota_t
    CF32 = KC_H2 + 3 * NT + 4             # ff2b_col|spans|muc|c2col|signbias
    OFF_W = KC_H * MPR                    # lwgwe chunks end / weffp start
    OFF_B = OFF_W + H                     # c1row | drow (row 0 cells)
    CBF = OFF_B + 2 * NL
    cf16 = inp("cf16", (P, CF16), F16)
    cbf = inp("cbf", (P, CBF), BF16)
    cf8 = inp("cf8", (P, KC_H2, 2, W8), F8)  # [ff2t8 | g65 | pad] per pair
    cf32 = inp("cf32", (P, CF32), F32)       # ff2b_col | spans
    we_t = inp("we_t", (P, KC_H, S), BF16)   # word_embedding[b].T chunked

    out = nc.dram_tensor("out", [NL, S], F32, kind="ExternalOutput").ap()

    with tile.TileContext(nc) as tc:
        with (
            tc.tile_pool(name="singles", bufs=1) as singles,
            tc.tile_pool(name="spans", bufs=4) as spans,
            tc.tile_pool(name="work", bufs=10) as work,
            tc.tile_pool(name="ps_cnt", bufs=1, space="PSUM") as ps_cnt,
            tc.tile_pool(name="ps_big", bufs=4, space="PSUM") as ps_big,
            tc.tile_pool(name="ps_pr", bufs=1, space="PSUM") as ps_pr,
            tc.tile_pool(name="ps_ss", bufs=1, space="PSUM") as ps_ss,
            tc.tile_pool(name="ps_corr", bufs=1, space="PSUM") as ps_corr,
        ):
            # ---- DMA queues: Pool = consts, SP = word embedding, Act = cbf
            cf32_sb = singles.tile([P, CF32], F32)
            nc.gpsimd.dma_start(out=cf32_sb, in_=cf32)
            cf16_sb = singles.tile([P, CF16], F16)
            nc.gpsimd.dma_start(out=cf16_sb, in_=cf16)
            cf8_sb = singles.tile([P, KC_H2, 2, W8], F8)
            nc.gpsimd.dma_start(out=cf8_sb, in_=cf8)
            cbf_sb = singles.tile([P, CBF], BF16)
            nc.sync.dma_start(out=cbf_sb, in_=cbf)
            we_sb = singles.tile([P, KC_H, S], BF16)
            nc.sync.dma_start(out=we_sb[:, 0:3, :], in_=we_t[:, 0:3, :])
            nc.sync.dma_start(out=we_sb[:, 3:6, :], in_=we_t[:, 3:6, :])

            iota_s = cf16_sb[:, 0:S]
            iota_t = cf16_sb[:, S:S + T]
            O_SPS, O_SPE, O_SPT = KC_H2, KC_H2 + NT, KC_H2 + 2 * NT
            O_MUB = KC_H2 + 3 * NT
            O_C2 = O_MUB + 1
            O_SGB, O_SLB = O_C2 + 1, O_C2 + 2
            c1row = cbf_sb[0:1, OFF_B:OFF_B + NL]
            drow = cbf_sb[0:1, OFF_B + NL:OFF_B + 2 * NL]

            ones_col = singles.tile([P, 1], BF16)
            nc.vector.memset(ones_col, 1.0)
            inv_col = singles.tile([P, 1], BF16)
            nc.vector.memset(inv_col, 1.0 / NEW_H)
            ones_row = singles.tile([1, NL], BF16)
            nc.vector.memset(ones_row, 1.0)
            ones_s = singles.tile([1, S], BF16)
            nc.vector.memset(ones_s, 1.0)
            eps_t = singles.tile([1, 1], F32)
            nc.vector.memset(eps_t, EPS)
            zero_t = singles.tile([1, 1], F32)
            nc.vector.memset(zero_t, 0.0)
            zcol = singles.tile([P, 1], F32)
            nc.vector.memset(zcol, 0.0)
            scratch = singles.tile([1, 1], F32)
            # one act-table set covers Relu + Square + Sqrt + Identity
            nc.scalar.activation(out=scratch, in_=eps_t, func=AF.Sqrt)

            # ---- counts: since start < end, covered = ge + lt - 1; the AND
            # is folded into the PE accumulation (2 matmuls per tile) with a
            # rank-1 correction (per-tag span count) subtracted afterwards.
            counts_ps = ps_cnt.tile([T, S], F32, tag="cnt")
            corr_ps = ps_corr.tile([T, 1], F32, tag="corr")
            for i in range(NT):
                lt_e = spans.tile([P, S], BF16, tag="lt_e")
                ge_s = spans.tile([P, S], BF16, tag="ge_s")
                nc.vector.tensor_scalar(
                    out=lt_e, in0=iota_s,
                    scalar1=cf32_sb[:, O_SPE + i:O_SPE + i + 1],
                    scalar2=None, op0=ALU.is_lt,
                )
                nc.vector.tensor_scalar(
                    out=ge_s, in0=iota_s,
                    scalar1=cf32_sb[:, O_SPS + i:O_SPS + i + 1],
                    scalar2=None, op0=ALU.is_ge,
                )
                onehot = spans.tile([P, T], BF16, tag="onehot")
                nc.vector.tensor_scalar(
                    out=onehot, in0=iota_t,
                    scalar1=cf32_sb[:, O_SPT + i:O_SPT + i + 1],
                    scalar2=None, op0=ALU.is_equal,
                )
                nc.tensor.matmul(
                    counts_ps, onehot, ge_s,
                    start=(i == 0), stop=False, skip_group_check=True,
                )
                nc.tensor.matmul(
                    counts_ps, onehot, lt_e,
                    start=False, stop=(i == NT - 1), skip_group_check=True,
                )
                nc.tensor.matmul(
                    corr_ps, onehot, ones_col,
                    start=(i == 0), stop=(i == NT - 1),
                )
            corr_sb = singles.tile([T, 1], F32)
            nc.scalar.activation(
                out=corr_sb, in_=corr_ps, func=AF.Identity,
                bias=zcol[0:T, :], scale=-1.0,
            )
            # counts + ones row -> [17, S] bf16 (counts are small ints: exact)
            # memset whole tile to 1.0 then overwrite rows 0:16 - row 16
            # keeps the 1.0 fill; the -corr add rides the psum->bf16 cast on
            # the Activation engine (keeps DVE free for masks/squares)
            counts_sb = singles.tile([T + 1, S], BF16)
            nc.vector.memset(counts_sb, 1.0)
            nc.scalar.activation(
                out=counts_sb[0:T, :], in_=counts_ps, func=AF.Identity,
                bias=corr_sb, scale=1.0,
            )

            # ---- pr psum: raw rows 0:33, sum row at 64 (32-aligned reads)
            pr = ps_pr.tile([MPR, S], F32, tag="pr")
            ss = ps_ss.tile([1, S], F32, tag="ss")

            # ---- h1 = relu(W_eff.T @ counts + b) -> r8 (fp8, 1/16) -------
            # h1 emitted BEFORE pr-we: the ready-order scheduler tie-breaks
            # on emission order, so h1-jc0 preempts the pr-we stream the
            # moment counts_sb lands and the relu wall starts earlier.
            HS = S // 2
            r8 = singles.tile([P, KC_H, S], F8)
            for jc in range(KC_H):
                h1 = ps_big.tile([P, S], F32, tag="big")
                nc.tensor.matmul(
                    h1,
                    cbf_sb[0:T + 1, OFF_W + jc * P:OFF_W + (jc + 1) * P],
                    counts_sb,
                    start=True, stop=True,
                )
                if jc % 2 == 0:
                    nc.scalar.activation(
                        out=r8[:, jc, :], in_=h1, func=AF.Relu,
                        bias=zcol, scale=1.0 / GSC,
                    )
                else:
                    nc.vector.tensor_scalar(
                        out=r8[:, jc, :], in0=h1, scalar1=1.0 / GSC,
                        scalar2=0.0, op0=ALU.mult, op1=ALU.max,
                    )

            # we part of raw/sum (6 bf16 matmuls) + squares for sumsq
            sqs = []
            for fc in range(KC_H):
                nc.tensor.matmul(
                    pr[0:MPR, :],
                    cbf_sb[:, fc * MPR:(fc + 1) * MPR],
                    we_sb[:, fc, :],
                    start=(fc == 0), stop=False,
                    skip_group_check=True,
                )
                sq = work.tile([P, S], BF16, tag="sq", name=f"sqwe{fc}")
                nc.vector.tensor_mul(
                    out=sq, in0=we_sb[:, fc, :], in1=we_sb[:, fc, :])
                sqs.append(sq)

            # fold the d (lwg_h2.T @ ff2_b) constant into the raw rows
            nc.tensor.matmul(
                pr[0:NL, :], drow, ones_s,
                start=False, stop=False, skip_group_check=True,
            )

            # G part of raw/sum: 3 fp8 DoubleRow matmuls into the same psum
            for pp in range(KC_H2):
                nc.tensor.matmul(
                    pr[0:MPR, :],
                    cf8_sb[:, pp, :, H2:H2 + MPR],
                    r8[:, 2 * pp:2 * pp + 2, :],
                    start=False, stop=False,
                    perf_mode=mybir.MatmulPerfMode.DoubleRow,
                    skip_group_check=True,
                )

            # mu right after the pr raw/sum rows are complete, then fold the
            # c1n * mu rank-1 term into the raw rows (closes the pr group).
            # mu on DVE (idle here; Act is busy with the h2 squares)
            mu = singles.tile([1, S], BF16)
            nc.vector.tensor_scalar(
                out=mu, in0=pr[MPR - 1:MPR, :], scalar1=1.0 / NEW_H,
                scalar2=cf32_sb[0:1, O_MUB:O_MUB + 1],
                op0=ALU.mult, op1=ALU.add,
            )
            mu2 = singles.tile([1, S], BF16)
            nc.vector.tensor_mul(out=mu2, in0=mu, in1=mu)

            # ---- h2 chunks (fp8 DR), then squares + sumsq matmuls --------
            h2ps = []
            for mc in range(KC_H2):
                h2 = ps_big.tile([P, S], F32, tag="big")
                for pp in range(KC_H2):
                    nc.tensor.matmul(
                        h2,
                        cf8_sb[:, pp, :, mc * P:(mc + 1) * P],
                        r8[:, 2 * pp:2 * pp + 2, :],
                        start=(pp == 0), stop=(pp == KC_H2 - 1),
                        perf_mode=mybir.MatmulPerfMode.DoubleRow,
                    )
                h2ps.append(h2)
            # ss-we matmuls ride here: PE is otherwise idle while the Act/DVE
            # squares drain, and they must precede the ss-h2 accumulations
            for fc in range(KC_H):
                nc.tensor.matmul(
                    ss, inv_col, sqs[fc],
                    start=(fc == 0), stop=False,
                    skip_group_check=True,
                )
            for mc in (0, 1, 2):
                sq = work.tile([P, S], BF16, tag="sq", name=f"sqh2{mc}")
                if mc == 2:
                    y = work.tile([P, S], BF16, tag="sq", name="ydve")
                    nc.vector.tensor_scalar(
                        out=y, in0=h2ps[mc], scalar1=GSC / FSC,
                        scalar2=cf32_sb[:, mc:mc + 1],
                        op0=ALU.mult, op1=ALU.add,
                    )
                    nc.vector.tensor_mul(out=sq, in0=y, in1=y)
                else:
                    nc.scalar.activation(
                        out=sq, in_=h2ps[mc], func=AF.Square,
                        bias=cf32_sb[:, mc:mc + 1], scale=GSC / FSC,
                    )
                nc.tensor.matmul(
                    ss, inv_col, sq,
                    start=False, stop=(mc == 2),
                    skip_group_check=True,
                )

            # fold the c1n * mu rank-1 term into the raw rows (closes pr)
            nc.tensor.matmul(
                pr[0:NL, :], c1row, mu,
                start=False, stop=True, skip_group_check=True,
            )
            # raw rows to SBUF (bf16): elementwise ops may read at most one
            # PSUM operand on real hardware.  Declared here, filled after the
            # h2 squares (Act is the only engine with slack there).
            praw = singles.tile([NL, S], BF16)

            nc.vector.tensor_copy(
                out=praw[:, 0:S // 2], in_=pr[0:NL, 0:S // 2])
            nc.scalar.activation(
                out=praw[:, S // 2:S], in_=pr[0:NL, S // 2:S],
                func=AF.Identity, bias=zcol[0:NL, :], scale=1.0,
            )

            # ---- LN stats ------------------------------------------------
            # ss already holds ex2 = sum(x^2)/NEW_H (1/N folded into lhsT);
            # pr rows 0:NL hold raw + d + c1n*mu, so the output is simply
            # rstd * pr + c2.
            # var -> sd -> 1/sd -> broadcast, pipelined in column halves
            # across DVE (var, recip) and Act (sqrt)
            var = singles.tile([1, S], BF16)
            sd = singles.tile([1, S], F32)
            rstd = singles.tile([1, S], BF16)
            rb_ps = ps_corr.tile([NL, S], F32, tag="corr")
            for h0, h1 in ((0, HS), (HS, S)):
                nc.vector.tensor_sub(
                    out=var[:, h0:h1], in0=ss[:, h0:h1], in1=mu2[:, h0:h1])
                nc.scalar.activation(
                    out=sd[:, h0:h1], in_=var[:, h0:h1], func=AF.Sqrt,
                    bias=eps_t, scale=1.0,
                )
                with nc.allow_low_precision(reason="bf16 rstd ample for LN"):
                    nc.vector.reciprocal(
                        out=rstd[:, h0:h1], in_=sd[:, h0:h1])
                nc.tensor.matmul(
                    rb_ps[:, h0:h1], ones_row, rstd[:, h0:h1],
                    start=True, stop=True,
                )

            # final = pr * rstd + c2, in column halves so each output DMA
            # launches as soon as its half is ready
            Q3 = HS
            f_sb = singles.tile([NL, S], F32)
            t2 = singles.tile([NL, S], BF16)
            nc.vector.tensor_mul(
                out=t2[:, 0:Q3], in0=praw[:, 0:Q3], in1=rb_ps[:, 0:Q3])
            nc.vector.tensor_scalar(
                out=f_sb[:, 0:Q3], in0=t2[:, 0:Q3],
                scalar1=cf32_sb[0:NL, O_C2:O_C2 + 1],
                scalar2=None, op0=ALU.add,
            )
            nc.sync.dma_start(out=out[:, 0:Q3], in_=f_sb[:, 0:Q3])
            nc.vector.tensor_mul(
                out=t2[:, Q3:S], in0=praw[:, Q3:S], in1=rb_ps[:, Q3:S])
            nc.vector.tensor_scalar(
                out=f_sb[:, Q3:S], in0=t2[:, Q3:S],
                scalar1=cf32_sb[0:NL, O_C2:O_C2 + 1],
                scalar2=None, op0=ALU.add,
            )
            nc.gpsimd.dma_start(out=out[:, Q3:S], in_=f_sb[:, Q3:S])

    nc.compile()
    return nc


_CACHE = {}


def kernel(**inputs) -> np.ndarray:
    bfl = ml_dtypes.bfloat16
    f8 = ml_dtypes.float8_e4m3
    we = np.asarray(inputs["word_embedding"], np.float32)
    te = np.asarray(inputs["tag_embedding"], np.float32)
    ipw = np.asarray(inputs["in_proj_w"], np.float32)
    ipb = np.asarray(inputs["in_proj_b"], np.float32)
    opw = np.asarray(inputs["out_proj_w"], np.float32)
    ob_ = np.asarray(inputs["out_proj_b"], np.float32)
    f1w = np.asarray(inputs["ff1_w"], np.float32)
    f1b = np.asarray(inputs["ff1_b"], np.float32)
    f2w = np.asarray(inputs["ff2_w"], np.float32)
    f2b = np.asarray(inputs["ff2_b"], np.float32)
    lg = np.asarray(inputs["ln_g"], np.float32)
    lb = np.asarray(inputs["ln_b"], np.float32)
    lw = np.asarray(inputs["lin_w"], np.float32)
    lbias = np.asarray(inputs["lin_b"], np.float32)
    sb = np.asarray(inputs["span_batch"]).astype(np.int64)
    st = np.asarray(inputs["span_tag"]).astype(np.int64)
    ss = np.asarray(inputs["span_start"]).astype(np.int64)
    se = np.asarray(inputs["span_end"]).astype(np.int64)

    # ---- host-side weight folding -------------------------------------
    v_tag = (te @ ipw[2 * H:].T + ipb[2 * H:]) @ opw.T + ob_       # [T, H]
    w_eff = np.einsum("th,jth->tj", v_tag, f1w.reshape(H, T, H))   # [T, H]
    weffp = np.concatenate([w_eff, f1b[None, :]], 0)               # [17, H]
    lwg = lw.T * lg[:, None]                                       # [NEW_H, NL]
    lwg_we, lwg_h2 = lwg[:H], lwg[H:]
    g_plus = np.zeros((H, 65), np.float32)                         # [H, 65]
    g_plus[:, :NL] = f2w.T @ lwg_h2
    g_plus[:, 64] = f2w.sum(0)
    g_plus *= GSC
    c1n = -lwg.sum(0)                                              # [NL]
    d = lwg_h2.T @ f2b                                             # [NL]
    c2 = lw @ lb + lbias                                           # [NL]
    sum_ff2b = float(f2b.sum())

    counts_per_b = np.bincount(sb, minlength=B)
    NT = max(1, int(np.ceil(counts_per_b.max() / P)))
    n_pad = NT * P

    # cbf: [128, 6*34 lwgwe+ones | H weffp | NL bw | 1 sum_ff2b]
    OFF_W = KC_H * MPR
    OFF_B = OFF_W + H
    CBF = OFF_B + 2 * NL
    cbf = np.zeros((P, CBF), np.float32)
    lwgwe_plus = np.zeros((H, 65), np.float32)                     # [H, 65]
    lwgwe_plus[:, :NL] = lwg_we
    lwgwe_plus[:, 64] = 1.0
    cbf[:, :OFF_W] = lwgwe_plus.reshape(KC_H, P, 65).transpose(
        1, 0, 2).reshape(P, OFF_W)
    cbf[0:T + 1, OFF_W:OFF_B] = weffp
    cbf[0, OFF_B:OFF_B + NL] = c1n
    cbf[0, OFF_B + NL:OFF_B + 2 * NL] = d
    cbf = cbf.astype(bfl)

    # cf8: per K-pair pp: [ff2t8(384) | g16(34) | pad] width 512
    cf8 = np.zeros((P, KC_H2, 2, W8), np.float32)
    ff2t8 = (f2w.T * FSC).reshape(KC_H2, 2, P, H2)                 # [3,2,128,H2]
    g16p = g_plus.reshape(KC_H2, 2, P, 65)                         # [3,2,128,65]
    cf8[:, :, :, 0:H2] = ff2t8.transpose(2, 0, 1, 3)
    cf8[:, :, :, H2:H2 + 65] = g16p.transpose(2, 0, 1, 3)
    cf8 = cf8.astype(f8)

    CF16 = S + T
    base16 = np.zeros((P, CF16), np.float16)
    base16[:, :S] = np.arange(S, dtype=np.float16)
    base16[:, S:S + T] = np.arange(T, dtype=np.float16)
    CF32 = KC_H2 + 3 * NT + 4
    base32 = np.zeros((P, CF32), np.float32)
    base32[:, :KC_H2] = f2b.reshape(KC_H2, P).T
    base32[0, KC_H2 + 3 * NT] = sum_ff2b / NEW_H
    base32[0:NL, KC_H2 + 3 * NT + 1] = c2

    in_maps = []
    for c in range(NCORES):
        idx = np.where(sb == c)[0]
        n = len(idx)
        spsv = np.zeros(n_pad, np.float32)
        spev = np.zeros(n_pad, np.float32)
        sptv = np.full(n_pad, -1.0, np.float32)   # pad tag -1: matches nothing
        spsv[:n] = ss[idx]
        spev[:n] = se[idx]
        sptv[:n] = st[idx]
        cf32_c = base32.copy()
        cf32_c[:, KC_H2 + 3 * NT + 2] = 0.5 - spsv[(NT - 1) * P:]
        cf32_c[:, KC_H2 + 3 * NT + 3] = spev[(NT - 1) * P:] - 0.5
        cf32_c[:, KC_H2:KC_H2 + NT] = spsv.reshape(NT, P).T
        cf32_c[:, KC_H2 + NT:KC_H2 + 2 * NT] = spev.reshape(NT, P).T
        cf32_c[:, KC_H2 + 2 * NT:KC_H2 + 3 * NT] = sptv.reshape(NT, P).T
        we_c = np.ascontiguousarray(we[c].T).reshape(KC_H, P, S)
        in_maps.append(dict(
            cf16=base16,
            cbf=cbf,
            cf8=cf8,
            cf32=cf32_c,
            we_t=np.ascontiguousarray(
                we_c.transpose(1, 0, 2)).astype(bfl),
        ))

    if NT not in _CACHE:
        _CACHE[NT] = build_kernel(NT)
    nc = _CACHE[NT]

    res = run_bass_kernel_spmd(nc, in_maps, list(range(NCORES)))
    out = np.stack([res.results[c]["out"].T for c in range(NCORES)])
    return out.astype(np.float32)


if __name__ == "__main__":
    import reference
    inp = {k: np.asarray(v) for k, v in reference.setup_inputs().items()}
    got = kernel(**inp)
    print("kernel output:", got.shape, got.dtype)

